# revision 21
# baseline (speedup 1.0000x reference)
"""Trainium2 Bass kernel for nn_Decoder (single-step LSTM + MDN head + sampling).

Strategy: data-parallel over batch across 8 NeuronCores (weights replicated,
batch sharded).  Everything is computed in "T orientation" (feature dim on
SBUF partitions, batch on the free dim) so LSTM gate activations can be fused
directly off PSUM and the projection accumulates into a single PSUM tile.

Two kernel variants are built lazily:
  * fast    — hidden/cell are all-zero (the spec fills them with zeros):
              gates = x @ W_ih.T + b, f-gate and cell term skipped entirely.
  * general — arbitrary hidden/cell: gates include hidden @ W_hh.T, the full
              i,f,g,o LSTM cell is evaluated.
Both share the same builder; the general variant simply has a longer
contraction dim (x ++ hidden ++ 1) and four gate blocks instead of three.

The MDN head (softmax, exp/tanh transforms, inverse-CDF component pick,
reparameterized gaussian sample, pen one-hot) runs fully on-device; the host
only shards inputs / concatenates outputs.
"""

import os
from contextlib import ExitStack

import numpy as np

import concourse.bacc as bacc
import concourse.bass as bass
import concourse.mybir as mybir
import concourse.tile as tile
from concourse.bass_utils import run_bass_kernel_spmd
from concourse.masks import make_identity

F32 = mybir.dt.float32
AF = mybir.ActivationFunctionType
OP = mybir.AluOpType

B, LATENT, STROKE, H, M = 2048, 128, 5, 2048, 20
NCORES = 8
BL = B // NCORES            # 256 batch rows per core
NH = H // 128               # 16 H tiles
NPROJ = 6 * M + 3           # 123

# packed output columns: [0:5 stroke_next][5:25 pi][25:45 mu_x][45:65 mu_y]
# [65:85 std_x][85:105 std_y][105:125 rho][125:128 q]
OUT_W = 128

_BUILT: dict = {}
LAST = None  # BassKernelResults of the most recent run (for test harness)


def _kt_sizes(kx):
    out = [128] * (kx // 128)
    if kx % 128:
        out.append(kx % 128)
    return out


def _build(general: bool, mm_dt=mybir.dt.float32r, stage="full"):
    """Build the Bass module for one variant. Returns nc.

    stage: "full" | "proj" (stop after projection) | "trans" (stop after
    transpose) — debug aid for localizing hardware runtime failures.
    """
    nc = bacc.Bacc("TRN2", target_bir_lowering=False, debug=False)

    if general:
        blocks = [("i", AF.Sigmoid), ("f", AF.Sigmoid), ("g", AF.Tanh), ("o", AF.Sigmoid)]
        KX = LATENT + STROKE + H + 1        # 2182
    else:
        blocks = [("i", AF.Sigmoid), ("g", AF.Tanh), ("o", AF.Sigmoid)]
        KX = LATENT + STROKE + 1            # 134
    NB = len(blocks)
    kts = _kt_sizes(KX)
    NKT = len(kts)
    kt_off = [sum(kts[:i]) for i in range(NKT)]

    xT_d = nc.dram_tensor("xT", [KX, BL], F32, kind="ExternalInput")
    wT_d = nc.dram_tensor("wT", [KX, NH * NB * 128], F32, kind="ExternalInput")
    wp_d = nc.dram_tensor("wprojT", [H + 1, NPROJ], F32, kind="ExternalInput")
    scal_d = nc.dram_tensor("scal", [128, 8], F32, kind="ExternalInput")
    if general:
        cT_d = nc.dram_tensor("cT", [H, BL], F32, kind="ExternalInput")
    out_d = nc.dram_tensor("out_pack", [BL, OUT_W], F32, kind="ExternalOutput")

    with tile.TileContext(nc) as tc, ExitStack() as ctx:
        pconst = ctx.enter_context(tc.tile_pool(name="const", bufs=1))
        pw = ctx.enter_context(tc.tile_pool(name="wstream", bufs=3))
        pgps = ctx.enter_context(tc.tile_pool(name="gpsum", bufs=4, space="PSUM"))
        ppso = ctx.enter_context(tc.tile_pool(name="opsum", bufs=1, space="PSUM"))
        ptrp = ctx.enter_context(tc.tile_pool(name="tpsum", bufs=2, space="PSUM"))
        pact = ctx.enter_context(tc.tile_pool(name="gact", bufs=2))
        pc = ctx.enter_context(tc.tile_pool(name="cell", bufs=2))
        ph = ctx.enter_context(tc.tile_pool(name="hidden", bufs=1))
        pct = ctx.enter_context(tc.tile_pool(name="ctin", bufs=3))
        phead = ctx.enter_context(tc.tile_pool(name="head", bufs=1))

        # ---- constants / persistent inputs ----
        # Matmul operands are typed mm_dt end-to-end (the BIR verifier
        # requires fp32r matmul inputs to be *produced* as fp32r).
        ident = pconst.tile([128, 128], F32, tag="ident")
        make_identity(nc, ident[:])
        ones_sb = pconst.tile([1, BL], mm_dt, tag="ones")
        nc.gpsimd.memset(ones_sb[:], 1.0)
        scal_sb = pconst.tile([128, 8], F32, tag="scal")
        nc.sync.dma_start(scal_sb[:], scal_d[:])

        xts = []
        for kt in range(NKT):
            xt = pconst.tile([kts[kt], BL], mm_dt, tag=f"xt{kt}")
            nc.sync.dma_start(
                xt[:], xT_d[kt_off[kt]:kt_off[kt] + kts[kt], :].bitcast(mm_dt))
            xts.append(xt)

        wps = []
        for k in range(NH):
            wp = pconst.tile([128, NPROJ], mm_dt, tag=f"wp{k}")
            nc.sync.dma_start(wp[:], wp_d[k * 128:(k + 1) * 128, :].bitcast(mm_dt))
            wps.append(wp)
        wp_last = pconst.tile([1, NPROJ], mm_dt, tag="wpL")
        nc.sync.dma_start(wp_last[:], wp_d[H:H + 1, :].bitcast(mm_dt))

        # ---- LSTM gates + cell update, one H-tile (128 units) at a time ----
        h_list = []
        for h in range(NH):
            wts = []
            for kt in range(NKT):
                wt = pw.tile([kts[kt], NB * 128], mm_dt, tag=f"w{kt}")
                nc.sync.dma_start(
                    wt[:],
                    wT_d[kt_off[kt]:kt_off[kt] + kts[kt],
                         h * NB * 128:(h + 1) * NB * 128].bitcast(mm_dt),
                )
                wts.append(wt)
            if general:
                ct_in = pct.tile([128, BL], F32, tag="ctin")
                nc.sync.dma_start(ct_in[:], cT_d[h * 128:(h + 1) * 128, :])

            sig_via_tanh = os.environ.get("KSIG", "lut") == "tanh"
            gates = {}
            for bi, (gname, func) in enumerate(blocks):
                ps = pgps.tile([128, BL], F32, tag="gps")
                for kt in range(NKT):
                    nc.tensor.matmul(
                        ps[:],
                        wts[kt][:, bi * 128:(bi + 1) * 128],
                        xts[kt][:],
                        start=(kt == 0),
                        stop=(kt == NKT - 1),
                    )
                g_sb = pact.tile([128, BL], F32, tag=f"g_{gname}")
                if func == AF.Sigmoid and sig_via_tanh:
                    # sigmoid(x) = 0.5*tanh(x/2) + 0.5 — tanh LUT is ~10x
                    # more accurate than the sigmoid LUT on ACT
                    nc.scalar.activation(g_sb[:], ps[:], AF.Tanh, scale=0.5)
                    nc.vector.tensor_scalar(g_sb[:], g_sb[:], 0.5, 0.5,
                                            op0=OP.mult, op1=OP.add)
                else:
                    nc.scalar.activation(g_sb[:], ps[:], func)
                gates[gname] = g_sb

            c_t = pc.tile([128, BL], F32, tag="c_new")
            if general:
                ig = pc.tile([128, BL], F32, tag="ig")
                fc = pc.tile([128, BL], F32, tag="fc")
                nc.vector.tensor_mul(ig[:], gates["i"][:], gates["g"][:])
                nc.vector.tensor_mul(fc[:], gates["f"][:], ct_in[:])
                nc.vector.tensor_add(c_t[:], fc[:], ig[:])
            else:
                nc.vector.tensor_mul(c_t[:], gates["i"][:], gates["g"][:])
            th = pc.tile([128, BL], F32, tag="tanh_c")
            nc.scalar.activation(th[:], c_t[:], AF.Tanh)
            h_sb = ph.tile([128, BL], mm_dt, tag=f"h{h}")
            nc.vector.tensor_mul(h_sb[:], gates["o"][:], th[:])
            h_list.append(h_sb)

        # ---- projection: outT (123, BL) accumulated over 16 H tiles + bias ----
        ps_out = ppso.tile([NPROJ, BL], F32, tag="pso")
        for k in range(NH):
            nc.tensor.matmul(
                ps_out[:], wps[k][:], h_list[k][:],
                start=(k == 0), stop=False,
            )
        nc.tensor.matmul(
            ps_out[:], wp_last[:], ones_sb[:],
            start=False, stop=True,
        )
        outT_sb = pconst.tile([NPROJ, BL], F32, tag="outT")
        nc.vector.tensor_copy(outT_sb[:], ps_out[:])

        if stage == "proj":
            out_flat = out_d.rearrange("a b -> (a b)")
            nc.sync.dma_start(
                out_flat[0:NPROJ * BL].rearrange("(p b) -> p b", b=BL),
                outT_sb[:])
            nc.compile()
            return nc

        # ---- MDN head, per batch tile of 128 rows ----
        for t in range(BL // 128):
            ps_tr = ptrp.tile([128, NPROJ], F32, tag="ptr")
            nc.tensor.transpose(
                ps_tr[:], outT_sb[:, t * 128:(t + 1) * 128], ident[:NPROJ, :NPROJ]
            )
            hd = phead.tile([128, NPROJ], F32, tag=f"hd{t}")
            nc.vector.tensor_copy(hd[:], ps_tr[:])

            if stage == "trans":
                nc.sync.dma_start(out_d[t * 128:(t + 1) * 128, 0:NPROJ], hd[:])
                continue

            out_sb = phead.tile([128, OUT_W], F32, tag=f"ob{t}")
            mixr = hd[:, 0:120].rearrange("p (j s) -> p s j", s=6)  # (128,6,20)

            def mv(k):
                return mixr[:, k, :]

            n1c = scal_sb[:, 4 * t + 0:4 * t + 1]
            n2c = scal_sb[:, 4 * t + 1:4 * t + 2]
            ucc = scal_sb[:, 4 * t + 2:4 * t + 3]
            upc = scal_sb[:, 4 * t + 3:4 * t + 4]

            def ht(shape, name):
                return phead.tile(shape, F32, name=f"{name}{t}", tag=f"{name}{t}")

            # pi = softmax(mix[...,0]); logits are tiny (~0.05) so no max-sub
            pi_e = ht([128, M], "pie")
            se = ht([128, 1], "se")
            nc.scalar.activation(pi_e[:], mv(0), AF.Exp, accum_out=se[:])
            rse = ht([128, 1], "rse")
            nc.vector.reciprocal(rse[:], se[:])
            nc.vector.tensor_scalar_mul(out_sb[:, 5:25], pi_e[:], rse[:])

            nc.vector.tensor_copy(out_sb[:, 25:45], mv(1))      # mu_x
            nc.vector.tensor_copy(out_sb[:, 45:65], mv(2))      # mu_y
            nc.scalar.activation(out_sb[:, 65:85], mv(3), AF.Exp)    # std_x
            nc.scalar.activation(out_sb[:, 85:105], mv(4), AF.Exp)   # std_y
            nc.scalar.activation(out_sb[:, 105:125], mv(5), AF.Tanh)  # rho

            # q = softmax(out[...,120:123])
            qe = ht([128, 3], "qe")
            sq = ht([128, 1], "sq")
            nc.scalar.activation(qe[:], hd[:, 120:123], AF.Exp, accum_out=sq[:])
            rsq = ht([128, 1], "rsq")
            nc.vector.reciprocal(rsq[:], sq[:])
            nc.vector.tensor_scalar_mul(out_sb[:, 125:128], qe[:], rsq[:])

            if stage == "h1":
                nc.gpsimd.memset(out_sb[:, 0:5], 0.0)
                nc.sync.dma_start(out_d[t * 128:(t + 1) * 128, :], out_sb[:])
                continue

            # cumsum(pi) via Hillis-Steele
            cA = ht([128, M], "cA")
            cB = ht([128, M], "cB")
            nc.vector.tensor_copy(cA[:], out_sb[:, 5:25])
            src, dst = cA, cB
            for k in (1, 2, 4, 8, 16):
                nc.vector.tensor_copy(dst[:, 0:k], src[:, 0:k])
                nc.vector.tensor_add(dst[:, k:M], src[:, k:M], src[:, 0:M - k])
                src, dst = dst, src
            cum = src

            # comp one-hot from the non-increasing indicator I = (cum < u)
            Ind = ht([128, M], "Ind")
            nc.vector.tensor_scalar(Ind[:], cum[:], ucc, None, op0=OP.is_lt)
            oh = ht([128, M], "oh")
            nc.vector.tensor_scalar(oh[:, 0:1], Ind[:, 0:1], -1.0, 1.0,
                                    op0=OP.mult, op1=OP.add)
            nc.vector.tensor_sub(oh[:, 1:M - 1], Ind[:, 0:M - 2], Ind[:, 1:M - 1])
            nc.vector.tensor_copy(oh[:, M - 1:M], Ind[:, M - 2:M - 1])

            if stage == "h2":
                nc.gpsimd.memset(out_sb[:, 0:5], 0.0)
                nc.vector.tensor_copy(out_sb[:, 25:45], oh[:])  # debug: oh
                nc.sync.dma_start(out_d[t * 128:(t + 1) * 128, :], out_sb[:])
                continue

            # gathers: take(a) = sum_j oh_j * a_j
            use_ttr = os.environ.get("KEXOTIC", "0") == "1"
            scr = ht([128, M], "scr")
            takes = {}
            for name, src_ap in (
                ("mx", out_sb[:, 25:45]), ("my", out_sb[:, 45:65]),
                ("sx", out_sb[:, 65:85]), ("sy", out_sb[:, 85:105]),
                ("r", out_sb[:, 105:125]), ("tv", mv(5)),
            ):
                dst_s = ht([128, 1], f"tk_{name}")
                if use_ttr:
                    nc.vector.tensor_tensor_reduce(
                        out=scr[:], in0=oh[:], in1=src_ap, scale=1.0, scalar=0.0,
                        op0=OP.mult, op1=OP.add, accum_out=dst_s[:],
                    )
                else:
                    nc.vector.tensor_mul(scr[:], oh[:], src_ap)
                    nc.vector.reduce_sum(dst_s[:], scr[:], axis=mybir.AxisListType.X)
                takes[name] = dst_s

            if stage == "h3":
                nc.gpsimd.memset(out_sb[:, 0:5], 0.0)
                nc.vector.tensor_copy(out_sb[:, 0:1], takes["mx"][:])
                nc.vector.tensor_copy(out_sb[:, 1:2], takes["r"][:])
                nc.sync.dma_start(out_d[t * 128:(t + 1) * 128, :], out_sb[:])
                continue

            # x_s = mx + sx * n1
            t1 = ht([128, 1], "t1")
            nc.vector.tensor_mul(t1[:], takes["sx"][:], n1c)
            nc.vector.tensor_add(out_sb[:, 0:1], takes["mx"][:], t1[:])

            # y_s = my + sy * (r*n1 + sech(v)*n2), sech(v) = sqrt(1 - tanh(v)^2)
            ev = ht([128, 1], "ev")
            nc.scalar.activation(ev[:], takes["tv"][:], AF.Exp)
            rev = ht([128, 1], "rev")
            nc.vector.reciprocal(rev[:], ev[:])
            den = ht([128, 1], "den")
            nc.vector.tensor_add(den[:], ev[:], rev[:])
            rden = ht([128, 1], "rden")
            nc.vector.reciprocal(rden[:], den[:])
            tt = ht([128, 1], "tt")
            nc.vector.tensor_mul(tt[:], rden[:], n2c)
            rn1 = ht([128, 1], "rn1")
            nc.vector.tensor_mul(rn1[:], takes["r"][:], n1c)
            inner = ht([128, 1], "inner")
            if os.environ.get("KEXOTIC", "0") == "1":
                nc.vector.scalar_tensor_tensor(
                    out=inner[:], in0=tt[:], scalar=2.0, in1=rn1[:],
                    op0=OP.mult, op1=OP.add,
                )
            else:
                tt2 = ht([128, 1], "tt2")
                nc.vector.tensor_scalar_mul(tt2[:], tt[:], 2.0)
                nc.vector.tensor_add(inner[:], tt2[:], rn1[:])
            t4 = ht([128, 1], "t4")
            nc.vector.tensor_mul(t4[:], takes["sy"][:], inner[:])
            nc.vector.tensor_add(out_sb[:, 1:2], takes["my"][:], t4[:])

            # pen one-hot from cumsum(q) < u_pen
            cq = ht([128, 3], "cq")
            nc.vector.tensor_copy(cq[:, 0:1], out_sb[:, 125:126])
            nc.vector.tensor_add(cq[:, 1:2], out_sb[:, 125:126], out_sb[:, 126:127])
            nc.vector.tensor_add(cq[:, 2:3], cq[:, 1:2], out_sb[:, 127:128])
            Iq = ht([128, 3], "Iq")
            nc.vector.tensor_scalar(Iq[:], cq[:], upc, None, op0=OP.is_lt)
            nc.vector.tensor_scalar(out_sb[:, 2:3], Iq[:, 0:1], -1.0, 1.0,
                                    op0=OP.mult, op1=OP.add)
            nc.vector.tensor_sub(out_sb[:, 3:4], Iq[:, 0:1], Iq[:, 1:2])
            nc.vector.tensor_copy(out_sb[:, 4:5], Iq[:, 1:2])

            nc.sync.dma_start(out_d[t * 128:(t + 1) * 128, :], out_sb[:])

    nc.compile()
    return nc


def _get_nc(general: bool):
    mm = os.environ.get("KDTYPE", "f32r")
    mm_dt = mybir.dt.float32r if mm == "f32r" else mybir.dt.float32
    key = (general, mm)
    if key not in _BUILT:
        _BUILT[key] = _build(general, mm_dt)
    return _BUILT[key]


def _tf32_round(a: np.ndarray) -> np.ndarray:
    """Round fp32 to TF32 (10-bit mantissa) with round-to-nearest-even.

    fp32r on the PE is TF32; pre-rounding on host makes the result
    independent of whether the hardware rounds or truncates on read.
    """
    u = a.view(np.uint32)
    r = (u + 0xFFF + ((u >> 13) & 1)) & np.uint32(0xFFFFE000)
    return r.view(np.float32)


def _host_inputs(general, z, stroke, hidden, cell, W_ih, W_hh, b_ih, b_hh,
                 W_proj, b_proj, n1, n2, u_comp, u_pen):
    """Build the per-core input maps (shared weight arrays, sharded batch)."""
    f = np.float32
    bias = (b_ih + b_hh).astype(f)
    if general:
        W_aug = np.concatenate(
            [W_ih, W_hh, bias[:, None]], axis=1, dtype=f)          # (8192, 2182)
        x_aug = np.concatenate(
            [z, stroke, hidden, np.ones((B, 1), f)], axis=1, dtype=f)
        block_ids = (0, 1, 2, 3)
    else:
        W_aug = np.concatenate([W_ih, bias[:, None]], axis=1, dtype=f)  # (8192, 134)
        x_aug = np.concatenate([z, stroke, np.ones((B, 1), f)], axis=1, dtype=f)
        block_ids = (0, 2, 3)   # i, g, o

    # interleave gate blocks by H tile: m-tile order [i_h, (f_h,) g_h, o_h]
    rows = []
    for h in range(NH):
        for b in block_ids:
            rows.append(np.arange(b * H + h * 128, b * H + (h + 1) * 128))
    rows = np.concatenate(rows)
    wT = np.ascontiguousarray(W_aug[rows].T)                  # (KX, NG*128)
    wpT = np.ascontiguousarray(
        np.concatenate([W_proj, b_proj[:, None]], axis=1, dtype=f).T)  # (2049,123)
    if os.environ.get("KDTYPE", "f32r") == "f32r":
        wT = _tf32_round(wT)
        wpT = _tf32_round(wpT)
        x_aug = _tf32_round(np.ascontiguousarray(x_aug))

    svals = np.stack([n1, n2, u_comp, u_pen], axis=-1).astype(f)  # (B, 4)

    in_maps = []
    for c in range(NCORES):
        sl = slice(c * BL, (c + 1) * BL)
        m = {
            "xT": np.ascontiguousarray(x_aug[sl].T),
            "wT": wT,
            "wprojT": wpT,
            "scal": np.ascontiguousarray(
                svals[sl].reshape(BL // 128, 128, 4).transpose(1, 0, 2)
                .reshape(128, 8)),
        }
        if general:
            m["cT"] = np.ascontiguousarray(cell[sl].T.astype(f))
        in_maps.append(m)
    return in_maps


def kernel(z, stroke, hidden, cell, W_ih, W_hh, b_ih, b_hh, W_proj, b_proj,
           n1, n2, u_comp, u_pen):
    global LAST
    args = [np.asarray(a, dtype=np.float32) for a in (
        z, stroke, hidden, cell, W_ih, W_hh, b_ih, b_hh, W_proj, b_proj,
        n1, n2, u_comp, u_pen)]
    (z, stroke, hidden, cell, W_ih, W_hh, b_ih, b_hh, W_proj, b_proj,
     n1, n2, u_comp, u_pen) = args

    general = bool(hidden.any() or cell.any())
    nc = _get_nc(general)
    in_maps = _host_inputs(general, z, stroke, hidden, cell, W_ih, W_hh,
                           b_ih, b_hh, W_proj, b_proj, n1, n2, u_comp, u_pen)

    trace = os.environ.get("KTRACE", "0") == "1"
    LAST = run_bass_kernel_spmd(nc, in_maps, list(range(NCORES)), trace=trace)
    out = np.concatenate([r["out_pack"] for r in LAST.results], axis=0)

    stroke_next = np.ascontiguousarray(out[:, 0:5])
    pi = np.ascontiguousarray(out[:, 5:25])
    mu_x = np.ascontiguousarray(out[:, 25:45])
    mu_y = np.ascontiguousarray(out[:, 45:65])
    std_x = np.ascontiguousarray(out[:, 65:85])
    std_y = np.ascontiguousarray(out[:, 85:105])
    rho = np.ascontiguousarray(out[:, 105:125])
    q = np.ascontiguousarray(out[:, 125:128])
    return (stroke_next, pi, mu_x, mu_y, std_x, std_y, rho, q)


# revision 27
# speedup vs baseline: 2.0474x; 2.0474x over previous
"""Trainium2 Bass kernel for nn_Decoder (single-step LSTM + MDN head + sampling).

Strategy: data-parallel over batch across 8 NeuronCores (weights replicated,
batch sharded).  Everything is computed in "T orientation" (feature dim on
SBUF partitions, batch on the free dim) so LSTM gate activations can be fused
directly off PSUM and the projection accumulates into a single PSUM tile.

Two kernel variants are built lazily:
  * fast    — hidden/cell are all-zero (the spec fills them with zeros):
              gates = x @ W_ih.T + b, f-gate and cell term skipped entirely.
  * general — arbitrary hidden/cell: gates include hidden @ W_hh.T, the full
              i,f,g,o LSTM cell is evaluated.
Both share the same builder; the general variant simply has a longer
contraction dim (x ++ hidden ++ 1) and four gate blocks instead of three.

The MDN head (softmax, exp/tanh transforms, inverse-CDF component pick,
reparameterized gaussian sample, pen one-hot) runs fully on-device; the host
only shards inputs / concatenates outputs.
"""

import os
from contextlib import ExitStack

import numpy as np

import concourse.bacc as bacc
import concourse.bass as bass
import concourse.mybir as mybir
import concourse.tile as tile
from concourse.bass_utils import run_bass_kernel_spmd
from concourse.masks import make_identity

F32 = mybir.dt.float32
AF = mybir.ActivationFunctionType
OP = mybir.AluOpType

B, LATENT, STROKE, H, M = 2048, 128, 5, 2048, 20
NCORES = 8
BL = B // NCORES            # 256 batch rows per core
NH = H // 128               # 16 H tiles
NPROJ = 6 * M + 3           # 123

# packed output columns: [0:5 stroke_next][5:25 pi][25:45 mu_x][45:65 mu_y]
# [65:85 std_x][85:105 std_y][105:125 rho][125:128 q]
OUT_W = 128

_BUILT: dict = {}
LAST = None  # BassKernelResults of the most recent run (for test harness)


def _kt_sizes(kx):
    out = [128] * (kx // 128)
    if kx % 128:
        out.append(kx % 128)
    return out


def _build(general: bool, mm_dt=mybir.dt.float32r, stage="full"):
    """Build the Bass module for one variant. Returns nc.

    stage: "full" | "proj" (stop after projection) | "trans" (stop after
    transpose) — debug aid for localizing hardware runtime failures.
    """
    nc = bacc.Bacc("TRN2", target_bir_lowering=False, debug=False)

    if general:
        blocks = [("i", AF.Sigmoid), ("f", AF.Sigmoid), ("g", AF.Tanh), ("o", AF.Sigmoid)]
        KX = LATENT + STROKE + H + 1        # 2182
    else:
        blocks = [("i", AF.Sigmoid), ("g", AF.Tanh), ("o", AF.Sigmoid)]
        KX = LATENT + STROKE + 1            # 134
    NB = len(blocks)
    kts = _kt_sizes(KX)
    NKT = len(kts)
    kt_off = [sum(kts[:i]) for i in range(NKT)]

    xT_d = nc.dram_tensor("xT", [KX, BL], F32, kind="ExternalInput")
    ones_d = nc.dram_tensor("ones", [1, BL], F32, kind="ExternalInput")
    wT_d = nc.dram_tensor("wT", [KX, NH * NB * 128], F32, kind="ExternalInput")
    wp_d = nc.dram_tensor("wprojT", [H + 1, NPROJ], F32, kind="ExternalInput")
    scal_d = nc.dram_tensor("scal", [128, 8], F32, kind="ExternalInput")
    if general:
        cT_d = nc.dram_tensor("cT", [H, BL], F32, kind="ExternalInput")
    out_d = nc.dram_tensor("out_pack", [BL, OUT_W], F32, kind="ExternalOutput")

    with tile.TileContext(nc) as tc, ExitStack() as ctx:
        pconst = ctx.enter_context(tc.tile_pool(name="const", bufs=1))
        pw = ctx.enter_context(tc.tile_pool(name="wstream", bufs=3))
        pgps = ctx.enter_context(tc.tile_pool(name="gpsum", bufs=4, space="PSUM"))
        ppso = ctx.enter_context(tc.tile_pool(name="opsum", bufs=1, space="PSUM"))
        ptrp = ctx.enter_context(tc.tile_pool(name="tpsum", bufs=2, space="PSUM"))
        pact = ctx.enter_context(tc.tile_pool(name="gact", bufs=2))
        pc = ctx.enter_context(tc.tile_pool(name="cell", bufs=2))
        ph = ctx.enter_context(tc.tile_pool(name="hidden", bufs=1))
        pct = ctx.enter_context(tc.tile_pool(name="ctin", bufs=3))
        phead = ctx.enter_context(tc.tile_pool(name="head", bufs=1))

        # ---- constants / persistent inputs ----
        # Matmul operands are typed mm_dt end-to-end (the BIR verifier
        # requires fp32r matmul inputs to be *produced* as fp32r).
        # DMA ordering matters: the gate-weight stream (sync engine) must
        # start immediately — everything only needed later (proj weights,
        # sampling scalars) goes on gpsimd queues.
        xts = []
        for kt in range(NKT):
            xt = pconst.tile([kts[kt], BL], mm_dt, tag=f"xt{kt}")
            nc.sync.dma_start(
                xt[:], xT_d[kt_off[kt]:kt_off[kt] + kts[kt], :].bitcast(mm_dt))
            xts.append(xt)

        ident = pconst.tile([128, 128], F32, tag="ident")
        make_identity(nc, ident[:])
        ones_sb = pconst.tile([1, BL], mm_dt, tag="ones")
        nc.gpsimd.dma_start(ones_sb[:], ones_d[:].bitcast(mm_dt))
        scal_sb = pconst.tile([128, 8], F32, tag="scal")
        nc.gpsimd.dma_start(scal_sb[:], scal_d[:])

        wps = []
        for k in range(NH):
            wp = pconst.tile([128, NPROJ], mm_dt, tag=f"wp{k}")
            nc.gpsimd.dma_start(wp[:], wp_d[k * 128:(k + 1) * 128, :].bitcast(mm_dt))
            wps.append(wp)
        wp_last = pconst.tile([1, NPROJ], mm_dt, tag="wpL")
        nc.gpsimd.dma_start(wp_last[:], wp_d[H:H + 1, :].bitcast(mm_dt))

        # ---- LSTM gates + cell update, one H-tile (128 units) at a time ----
        h_list = []
        for h in range(NH):
            wts = []
            for kt in range(NKT):
                wt = pw.tile([kts[kt], NB * 128], mm_dt, tag=f"w{kt}")
                nc.sync.dma_start(
                    wt[:],
                    wT_d[kt_off[kt]:kt_off[kt] + kts[kt],
                         h * NB * 128:(h + 1) * NB * 128].bitcast(mm_dt),
                )
                wts.append(wt)
            if general:
                ct_in = pct.tile([128, BL], F32, tag="ctin")
                nc.sync.dma_start(ct_in[:], cT_d[h * 128:(h + 1) * 128, :])

            sig_via_tanh = os.environ.get("KSIG", "lut") == "tanh"
            gates = {}
            for bi, (gname, func) in enumerate(blocks):
                ps = pgps.tile([128, BL], F32, tag="gps")
                for kt in range(NKT):
                    nc.tensor.matmul(
                        ps[:],
                        wts[kt][:, bi * 128:(bi + 1) * 128],
                        xts[kt][:],
                        start=(kt == 0),
                        stop=(kt == NKT - 1),
                    )
                g_sb = pact.tile([128, BL], F32, tag=f"g_{gname}")
                if func == AF.Sigmoid and sig_via_tanh:
                    # sigmoid(x) = 0.5*tanh(x/2) + 0.5 — tanh LUT is ~10x
                    # more accurate than the sigmoid LUT on ACT
                    nc.scalar.activation(g_sb[:], ps[:], AF.Tanh, scale=0.5)
                    nc.vector.tensor_scalar(g_sb[:], g_sb[:], 0.5, 0.5,
                                            op0=OP.mult, op1=OP.add)
                else:
                    nc.scalar.activation(g_sb[:], ps[:], func)
                gates[gname] = g_sb

            c_t = pc.tile([128, BL], F32, tag="c_new")
            if general:
                ig = pc.tile([128, BL], F32, tag="ig")
                fc = pc.tile([128, BL], F32, tag="fc")
                nc.vector.tensor_mul(ig[:], gates["i"][:], gates["g"][:])
                nc.vector.tensor_mul(fc[:], gates["f"][:], ct_in[:])
                nc.vector.tensor_add(c_t[:], fc[:], ig[:])
            else:
                nc.vector.tensor_mul(c_t[:], gates["i"][:], gates["g"][:])
            th = pc.tile([128, BL], F32, tag="tanh_c")
            nc.scalar.activation(th[:], c_t[:], AF.Tanh)
            h_sb = ph.tile([128, BL], mm_dt, tag=f"h{h}")
            nc.vector.tensor_mul(h_sb[:], gates["o"][:], th[:])
            h_list.append(h_sb)

        # ---- projection: outT (123, BL) accumulated over 16 H tiles + bias ----
        ps_out = ppso.tile([NPROJ, BL], F32, tag="pso")
        for k in range(NH):
            nc.tensor.matmul(
                ps_out[:], wps[k][:], h_list[k][:],
                start=(k == 0), stop=False,
            )
        nc.tensor.matmul(
            ps_out[:], wp_last[:], ones_sb[:],
            start=False, stop=True,
        )
        outT_sb = pconst.tile([NPROJ, BL], F32, tag="outT")
        nc.vector.tensor_copy(outT_sb[:], ps_out[:])

        if stage == "proj":
            out_flat = out_d.rearrange("a b -> (a b)")
            nc.sync.dma_start(
                out_flat[0:NPROJ * BL].rearrange("(p b) -> p b", b=BL),
                outT_sb[:])
            nc.compile()
            return nc

        # ---- MDN head: both batch tiles combined as (128, 2, ...) ----
        NT = BL // 128  # 2
        head2 = phead.tile([128, NT, NPROJ], F32, tag="head2")
        for t in range(NT):
            ps_tr = ptrp.tile([128, NPROJ], F32, tag="ptr")
            nc.tensor.transpose(
                ps_tr[:], outT_sb[:, t * 128:(t + 1) * 128], ident[:NPROJ, :NPROJ]
            )
            nc.vector.tensor_copy(head2[:, t, :], ps_tr[:])

        if stage == "trans":
            for t in range(NT):
                nc.sync.dma_start(out_d[t * 128:(t + 1) * 128, 0:NPROJ],
                                  head2[:, t, :])
            nc.compile()
            return nc

        out2 = phead.tile([128, NT, OUT_W], F32, tag="out2")

        # strided views over the interleaved MDN params, both tiles at once
        mix4 = head2[:, :, 0:120].rearrange("p t (j s) -> p s t j", s=6)

        def mv2(k):
            return mix4[:, k, :, :]          # (128, NT, M)

        def oc(k):
            return out2[:, :, k]             # (128, NT)

        n1b = scal_sb[:, 0:NT]
        n2b = scal_sb[:, NT:2 * NT]
        ucb = scal_sb[:, 2 * NT:3 * NT]
        upb = scal_sb[:, 3 * NT:4 * NT]

        def ht(shape, name):
            return phead.tile(shape, F32, name=name, tag=name)

        # pi softmax (logits tiny -> no max subtraction needed)
        pi_e = ht([128, NT, M], "pie")
        nc.scalar.activation(pi_e[:], mv2(0), AF.Exp)
        se = ht([128, NT], "se")
        nc.vector.reduce_sum(se[:], pi_e[:], axis=mybir.AxisListType.X)
        rse = ht([128, NT], "rse")
        nc.vector.reciprocal(rse[:], se[:])
        for t in range(NT):
            nc.vector.tensor_scalar_mul(out2[:, t, 5:25], pi_e[:, t, :],
                                        rse[:, t:t + 1])

        nc.vector.tensor_copy(out2[:, :, 25:45], mv2(1))        # mu_x
        nc.vector.tensor_copy(out2[:, :, 45:65], mv2(2))        # mu_y
        nc.scalar.activation(out2[:, :, 65:85], mv2(3), AF.Exp)     # std_x
        nc.scalar.activation(out2[:, :, 85:105], mv2(4), AF.Exp)    # std_y
        nc.scalar.activation(out2[:, :, 105:125], mv2(5), AF.Tanh)  # rho

        # q softmax
        qe = ht([128, NT, 3], "qe")
        nc.scalar.activation(qe[:], head2[:, :, 120:123], AF.Exp)
        sq = ht([128, NT], "sq")
        nc.vector.reduce_sum(sq[:], qe[:], axis=mybir.AxisListType.X)
        rsq = ht([128, NT], "rsq")
        nc.vector.reciprocal(rsq[:], sq[:])
        for t in range(NT):
            nc.vector.tensor_scalar_mul(out2[:, t, 125:128], qe[:, t, :],
                                        rsq[:, t:t + 1])

        # component pick: cumsum(e) < u * sum(e)  (same predicate as
        # cumsum(softmax) < u, fewer roundings)
        cA = ht([128, NT, M], "cA")
        cB = ht([128, NT, M], "cB")
        nc.vector.tensor_copy(cA[:], pi_e[:])
        src, dst = cA, cB
        for k in (1, 2, 4, 8, 16):
            nc.vector.tensor_copy(dst[:, :, 0:k], src[:, :, 0:k])
            nc.vector.tensor_add(dst[:, :, k:M], src[:, :, k:M],
                                 src[:, :, 0:M - k])
            src, dst = dst, src
        cum = src
        use_ = ht([128, NT], "use_")
        nc.vector.tensor_mul(use_[:], ucb, se[:])

        Ind = ht([128, NT, M], "Ind")
        for t in range(NT):
            nc.vector.tensor_scalar(Ind[:, t, :], cum[:, t, :],
                                    use_[:, t:t + 1], None, op0=OP.is_lt)
        oh = ht([128, NT, M], "oh")
        nc.vector.tensor_scalar(oh[:, :, 0:1], Ind[:, :, 0:1], -1.0, 1.0,
                                op0=OP.mult, op1=OP.add)
        nc.vector.tensor_sub(oh[:, :, 1:M - 1], Ind[:, :, 0:M - 2],
                             Ind[:, :, 1:M - 1])
        nc.vector.tensor_copy(oh[:, :, M - 1:M], Ind[:, :, M - 2:M - 1])

        # gathers: take(a) = sum_j oh_j * a_j  -> (128, NT)
        takes = {}
        for name, src_ap in (
            ("mx", out2[:, :, 25:45]), ("my", out2[:, :, 45:65]),
            ("sx", out2[:, :, 65:85]), ("sy", out2[:, :, 85:105]),
            ("r", out2[:, :, 105:125]), ("tv", mv2(5)),
        ):
            scr = ht([128, NT, M], f"scr_{name}")
            dst_s = ht([128, NT], f"tk_{name}")
            nc.vector.tensor_mul(scr[:], oh[:], src_ap)
            nc.vector.reduce_sum(dst_s[:], scr[:], axis=mybir.AxisListType.X)
            takes[name] = dst_s

        # x_s = mx + sx * n1
        t1 = ht([128, NT], "t1")
        nc.vector.tensor_mul(t1[:], takes["sx"][:], n1b)
        nc.vector.tensor_add(oc(0), takes["mx"][:], t1[:])

        # y_s = my + sy * (r*n1 + sech(v)*n2), sech(v) = sqrt(1 - tanh(v)^2)
        ev = ht([128, NT], "ev")
        nc.scalar.activation(ev[:], takes["tv"][:], AF.Exp)
        rev = ht([128, NT], "rev")
        nc.vector.reciprocal(rev[:], ev[:])
        den = ht([128, NT], "den")
        nc.vector.tensor_add(den[:], ev[:], rev[:])
        rden = ht([128, NT], "rden")
        nc.vector.reciprocal(rden[:], den[:])
        tt = ht([128, NT], "tt")
        nc.vector.tensor_mul(tt[:], rden[:], n2b)
        tt2 = ht([128, NT], "tt2")
        nc.vector.tensor_scalar_mul(tt2[:], tt[:], 2.0)
        rn1 = ht([128, NT], "rn1")
        nc.vector.tensor_mul(rn1[:], takes["r"][:], n1b)
        inner = ht([128, NT], "inner")
        nc.vector.tensor_add(inner[:], tt2[:], rn1[:])
        t4 = ht([128, NT], "t4")
        nc.vector.tensor_mul(t4[:], takes["sy"][:], inner[:])
        nc.vector.tensor_add(oc(1), takes["my"][:], t4[:])

        # pen one-hot from cumsum(q) < u_pen
        cq = ht([128, NT, 3], "cq")
        nc.vector.tensor_copy(cq[:, :, 0:1], out2[:, :, 125:126])
        nc.vector.tensor_add(cq[:, :, 1:2], out2[:, :, 125:126],
                             out2[:, :, 126:127])
        nc.vector.tensor_add(cq[:, :, 2:3], cq[:, :, 1:2], out2[:, :, 127:128])
        Iq = ht([128, NT, 3], "Iq")
        for t in range(NT):
            nc.vector.tensor_scalar(Iq[:, t, :], cq[:, t, :], upb[:, t:t + 1],
                                    None, op0=OP.is_lt)
        nc.vector.tensor_scalar(out2[:, :, 2:3], Iq[:, :, 0:1], -1.0, 1.0,
                                op0=OP.mult, op1=OP.add)
        nc.vector.tensor_sub(out2[:, :, 3:4], Iq[:, :, 0:1], Iq[:, :, 1:2])
        nc.vector.tensor_copy(out2[:, :, 4:5], Iq[:, :, 1:2])

        nc.sync.dma_start(
            out_d.rearrange("(t p) c -> p t c", p=128), out2[:])

    nc.compile()
    return nc


def _get_nc(general: bool):
    mm = os.environ.get("KDTYPE", "f32r")
    mm_dt = mybir.dt.float32r if mm == "f32r" else mybir.dt.float32
    key = (general, mm)
    if key not in _BUILT:
        _BUILT[key] = _build(general, mm_dt)
    return _BUILT[key]


def _tf32_round(a: np.ndarray) -> np.ndarray:
    """Round fp32 to TF32 (10-bit mantissa) with round-to-nearest-even.

    fp32r on the PE is TF32; pre-rounding on host makes the result
    independent of whether the hardware rounds or truncates on read.
    """
    u = a.view(np.uint32)
    r = (u + 0xFFF + ((u >> 13) & 1)) & np.uint32(0xFFFFE000)
    return r.view(np.float32)


def _host_inputs(general, z, stroke, hidden, cell, W_ih, W_hh, b_ih, b_hh,
                 W_proj, b_proj, n1, n2, u_comp, u_pen):
    """Build the per-core input maps (shared weight arrays, sharded batch)."""
    f = np.float32
    bias = (b_ih + b_hh).astype(f)
    if general:
        W_aug = np.concatenate(
            [W_ih, W_hh, bias[:, None]], axis=1, dtype=f)          # (8192, 2182)
        x_aug = np.concatenate(
            [z, stroke, hidden, np.ones((B, 1), f)], axis=1, dtype=f)
        block_ids = (0, 1, 2, 3)
    else:
        W_aug = np.concatenate([W_ih, bias[:, None]], axis=1, dtype=f)  # (8192, 134)
        x_aug = np.concatenate([z, stroke, np.ones((B, 1), f)], axis=1, dtype=f)
        block_ids = (0, 2, 3)   # i, g, o

    # interleave gate blocks by H tile: m-tile order [i_h, (f_h,) g_h, o_h]
    rows = []
    for h in range(NH):
        for b in block_ids:
            rows.append(np.arange(b * H + h * 128, b * H + (h + 1) * 128))
    rows = np.concatenate(rows)
    wT = np.ascontiguousarray(W_aug[rows].T)                  # (KX, NG*128)
    wpT = np.ascontiguousarray(
        np.concatenate([W_proj, b_proj[:, None]], axis=1, dtype=f).T)  # (2049,123)
    if os.environ.get("KDTYPE", "f32r") == "f32r":
        wT = _tf32_round(wT)
        wpT = _tf32_round(wpT)
        x_aug = _tf32_round(np.ascontiguousarray(x_aug))

    svals = np.stack([n1, n2, u_comp, u_pen], axis=-1).astype(f)  # (B, 4)

    in_maps = []
    for c in range(NCORES):
        sl = slice(c * BL, (c + 1) * BL)
        m = {
            "xT": np.ascontiguousarray(x_aug[sl].T),
            "ones": np.ones((1, BL), f),
            "wT": wT,
            "wprojT": wpT,
            # scal[p, k, t]: k in (n1, n2, u_comp, u_pen), t = batch tile
            "scal": np.ascontiguousarray(
                svals[sl].reshape(BL // 128, 128, 4).transpose(1, 2, 0)
                .reshape(128, 8)),
        }
        if general:
            m["cT"] = np.ascontiguousarray(cell[sl].T.astype(f))
        in_maps.append(m)
    return in_maps


def kernel(z, stroke, hidden, cell, W_ih, W_hh, b_ih, b_hh, W_proj, b_proj,
           n1, n2, u_comp, u_pen):
    global LAST
    args = [np.asarray(a, dtype=np.float32) for a in (
        z, stroke, hidden, cell, W_ih, W_hh, b_ih, b_hh, W_proj, b_proj,
        n1, n2, u_comp, u_pen)]
    (z, stroke, hidden, cell, W_ih, W_hh, b_ih, b_hh, W_proj, b_proj,
     n1, n2, u_comp, u_pen) = args

    general = bool(hidden.any() or cell.any())
    nc = _get_nc(general)
    in_maps = _host_inputs(general, z, stroke, hidden, cell, W_ih, W_hh,
                           b_ih, b_hh, W_proj, b_proj, n1, n2, u_comp, u_pen)

    trace = os.environ.get("KTRACE", "0") == "1"
    LAST = run_bass_kernel_spmd(nc, in_maps, list(range(NCORES)), trace=trace)
    out = np.concatenate([r["out_pack"] for r in LAST.results], axis=0)

    stroke_next = np.ascontiguousarray(out[:, 0:5])
    pi = np.ascontiguousarray(out[:, 5:25])
    mu_x = np.ascontiguousarray(out[:, 25:45])
    mu_y = np.ascontiguousarray(out[:, 45:65])
    std_x = np.ascontiguousarray(out[:, 65:85])
    std_y = np.ascontiguousarray(out[:, 85:105])
    rho = np.ascontiguousarray(out[:, 105:125])
    q = np.ascontiguousarray(out[:, 125:128])
    return (stroke_next, pi, mu_x, mu_y, std_x, std_y, rho, q)


# revision 28
# speedup vs baseline: 2.0722x; 1.0121x over previous
"""Trainium2 Bass kernel for nn_Decoder (single-step LSTM + MDN head + sampling).

Strategy: data-parallel over batch across 8 NeuronCores (weights replicated,
batch sharded).  Everything is computed in "T orientation" (feature dim on
SBUF partitions, batch on the free dim) so LSTM gate activations can be fused
directly off PSUM and the projection accumulates into a single PSUM tile.

Two kernel variants are built lazily:
  * fast    — hidden/cell are all-zero (the spec fills them with zeros):
              gates = x @ W_ih.T + b, f-gate and cell term skipped entirely.
  * general — arbitrary hidden/cell: gates include hidden @ W_hh.T, the full
              i,f,g,o LSTM cell is evaluated.
Both share the same builder; the general variant simply has a longer
contraction dim (x ++ hidden ++ 1) and four gate blocks instead of three.

The MDN head (softmax, exp/tanh transforms, inverse-CDF component pick,
reparameterized gaussian sample, pen one-hot) runs fully on-device; the host
only shards inputs / concatenates outputs.
"""

import os
from contextlib import ExitStack

import numpy as np

import concourse.bacc as bacc
import concourse.bass as bass
import concourse.mybir as mybir
import concourse.tile as tile
from concourse.bass_utils import run_bass_kernel_spmd
from concourse.masks import make_identity

F32 = mybir.dt.float32
AF = mybir.ActivationFunctionType
OP = mybir.AluOpType

B, LATENT, STROKE, H, M = 2048, 128, 5, 2048, 20
NCORES = 8
BL = B // NCORES            # 256 batch rows per core
NH = H // 128               # 16 H tiles
NPROJ = 6 * M + 3           # 123

# packed output columns: [0:5 stroke_next][5:25 pi][25:45 mu_x][45:65 mu_y]
# [65:85 std_x][85:105 std_y][105:125 rho][125:128 q]
OUT_W = 128

_BUILT: dict = {}
LAST = None  # BassKernelResults of the most recent run (for test harness)


def _kt_sizes(kx):
    out = [128] * (kx // 128)
    if kx % 128:
        out.append(kx % 128)
    return out


def _build(general: bool, mm_dt=mybir.dt.float32r, stage="full"):
    """Build the Bass module for one variant. Returns nc.

    stage: "full" | "proj" (stop after projection) | "trans" (stop after
    transpose) — debug aid for localizing hardware runtime failures.
    """
    nc = bacc.Bacc("TRN2", target_bir_lowering=False, debug=False)

    if general:
        blocks = [("i", AF.Sigmoid), ("f", AF.Sigmoid), ("g", AF.Tanh), ("o", AF.Sigmoid)]
        KX = LATENT + STROKE + H + 1        # 2182
    else:
        blocks = [("i", AF.Sigmoid), ("g", AF.Tanh), ("o", AF.Sigmoid)]
        KX = LATENT + STROKE + 1            # 134
    NB = len(blocks)
    kts = _kt_sizes(KX)
    NKT = len(kts)
    kt_off = [sum(kts[:i]) for i in range(NKT)]

    xT_d = nc.dram_tensor("xT", [KX, BL], F32, kind="ExternalInput")
    ones_d = nc.dram_tensor("ones", [1, BL], F32, kind="ExternalInput")
    wT_d = nc.dram_tensor("wT", [KX, NH * NB * 128], F32, kind="ExternalInput")
    wp_d = nc.dram_tensor("wprojT", [H + 1, NPROJ], F32, kind="ExternalInput")
    scal_d = nc.dram_tensor("scal", [128, 8], F32, kind="ExternalInput")
    if general:
        cT_d = nc.dram_tensor("cT", [H, BL], F32, kind="ExternalInput")
    out_d = nc.dram_tensor("out_pack", [BL, OUT_W], F32, kind="ExternalOutput")

    with tile.TileContext(nc) as tc, ExitStack() as ctx:
        pconst = ctx.enter_context(tc.tile_pool(name="const", bufs=1))
        pw = ctx.enter_context(tc.tile_pool(name="wstream", bufs=4))
        pgps = ctx.enter_context(tc.tile_pool(name="gpsum", bufs=5, space="PSUM"))
        ppso = ctx.enter_context(tc.tile_pool(name="opsum", bufs=1, space="PSUM"))
        ptrp = ctx.enter_context(tc.tile_pool(name="tpsum", bufs=2, space="PSUM"))
        pact = ctx.enter_context(tc.tile_pool(name="gact", bufs=2))
        pc = ctx.enter_context(tc.tile_pool(name="cell", bufs=2))
        ph = ctx.enter_context(tc.tile_pool(name="hidden", bufs=1))
        pct = ctx.enter_context(tc.tile_pool(name="ctin", bufs=3))
        phead = ctx.enter_context(tc.tile_pool(name="head", bufs=1))

        # ---- constants / persistent inputs ----
        # Matmul operands are typed mm_dt end-to-end (the BIR verifier
        # requires fp32r matmul inputs to be *produced* as fp32r).
        # DMA ordering matters: the gate-weight stream (sync engine) must
        # start immediately — everything only needed later (proj weights,
        # sampling scalars) goes on gpsimd queues.
        xts = []
        for kt in range(NKT):
            xt = pconst.tile([kts[kt], BL], mm_dt, tag=f"xt{kt}")
            nc.sync.dma_start(
                xt[:], xT_d[kt_off[kt]:kt_off[kt] + kts[kt], :].bitcast(mm_dt))
            xts.append(xt)

        ident = pconst.tile([128, 128], F32, tag="ident")
        make_identity(nc, ident[:])
        ones_sb = pconst.tile([1, BL], mm_dt, tag="ones")
        nc.gpsimd.dma_start(ones_sb[:], ones_d[:].bitcast(mm_dt))
        scal_sb = pconst.tile([128, 8], F32, tag="scal")
        nc.gpsimd.dma_start(scal_sb[:], scal_d[:])

        wps = []
        for k in range(NH):
            wp = pconst.tile([128, NPROJ], mm_dt, tag=f"wp{k}")
            nc.gpsimd.dma_start(wp[:], wp_d[k * 128:(k + 1) * 128, :].bitcast(mm_dt))
            wps.append(wp)
        wp_last = pconst.tile([1, NPROJ], mm_dt, tag="wpL")
        nc.gpsimd.dma_start(wp_last[:], wp_d[H:H + 1, :].bitcast(mm_dt))

        # ---- LSTM gates + cell update, one H-tile (128 units) at a time ----
        h_list = []
        for h in range(NH):
            wts = []
            for kt in range(NKT):
                wt = pw.tile([kts[kt], NB * 128], mm_dt, tag=f"w{kt}")
                nc.sync.dma_start(
                    wt[:],
                    wT_d[kt_off[kt]:kt_off[kt] + kts[kt],
                         h * NB * 128:(h + 1) * NB * 128].bitcast(mm_dt),
                )
                wts.append(wt)
            if general:
                ct_in = pct.tile([128, BL], F32, tag="ctin")
                nc.sync.dma_start(ct_in[:], cT_d[h * 128:(h + 1) * 128, :])

            sig_via_tanh = os.environ.get("KSIG", "lut") == "tanh"
            gates = {}
            for bi, (gname, func) in enumerate(blocks):
                ps = pgps.tile([128, BL], F32, tag="gps")
                for kt in range(NKT):
                    nc.tensor.matmul(
                        ps[:],
                        wts[kt][:, bi * 128:(bi + 1) * 128],
                        xts[kt][:],
                        start=(kt == 0),
                        stop=(kt == NKT - 1),
                    )
                g_sb = pact.tile([128, BL], F32, tag=f"g_{gname}")
                if func == AF.Sigmoid and sig_via_tanh:
                    # sigmoid(x) = 0.5*tanh(x/2) + 0.5 — tanh LUT is ~10x
                    # more accurate than the sigmoid LUT on ACT
                    nc.scalar.activation(g_sb[:], ps[:], AF.Tanh, scale=0.5)
                    nc.vector.tensor_scalar(g_sb[:], g_sb[:], 0.5, 0.5,
                                            op0=OP.mult, op1=OP.add)
                else:
                    nc.scalar.activation(g_sb[:], ps[:], func)
                gates[gname] = g_sb

            c_t = pc.tile([128, BL], F32, tag="c_new")
            if general:
                ig = pc.tile([128, BL], F32, tag="ig")
                fc = pc.tile([128, BL], F32, tag="fc")
                nc.vector.tensor_mul(ig[:], gates["i"][:], gates["g"][:])
                nc.vector.tensor_mul(fc[:], gates["f"][:], ct_in[:])
                nc.vector.tensor_add(c_t[:], fc[:], ig[:])
            else:
                nc.vector.tensor_mul(c_t[:], gates["i"][:], gates["g"][:])
            th = pc.tile([128, BL], F32, tag="tanh_c")
            nc.scalar.activation(th[:], c_t[:], AF.Tanh)
            h_sb = ph.tile([128, BL], mm_dt, tag=f"h{h}")
            nc.vector.tensor_mul(h_sb[:], gates["o"][:], th[:])
            h_list.append(h_sb)

        # ---- projection: outT (123, BL) accumulated over 16 H tiles + bias ----
        ps_out = ppso.tile([NPROJ, BL], F32, tag="pso")
        for k in range(NH):
            nc.tensor.matmul(
                ps_out[:], wps[k][:], h_list[k][:],
                start=(k == 0), stop=False,
            )
        nc.tensor.matmul(
            ps_out[:], wp_last[:], ones_sb[:],
            start=False, stop=True,
        )
        outT_sb = pconst.tile([NPROJ, BL], F32, tag="outT")
        nc.vector.tensor_copy(outT_sb[:], ps_out[:])

        if stage == "proj":
            out_flat = out_d.rearrange("a b -> (a b)")
            nc.sync.dma_start(
                out_flat[0:NPROJ * BL].rearrange("(p b) -> p b", b=BL),
                outT_sb[:])
            nc.compile()
            return nc

        # ---- MDN head: both batch tiles combined as (128, 2, ...) ----
        NT = BL // 128  # 2
        head2 = phead.tile([128, NT, NPROJ], F32, tag="head2")
        for t in range(NT):
            ps_tr = ptrp.tile([128, NPROJ], F32, tag="ptr")
            nc.tensor.transpose(
                ps_tr[:], outT_sb[:, t * 128:(t + 1) * 128], ident[:NPROJ, :NPROJ]
            )
            nc.vector.tensor_copy(head2[:, t, :], ps_tr[:])

        if stage == "trans":
            for t in range(NT):
                nc.sync.dma_start(out_d[t * 128:(t + 1) * 128, 0:NPROJ],
                                  head2[:, t, :])
            nc.compile()
            return nc

        out2 = phead.tile([128, NT, OUT_W], F32, tag="out2")

        # strided views over the interleaved MDN params, both tiles at once
        mix4 = head2[:, :, 0:120].rearrange("p t (j s) -> p s t j", s=6)

        def mv2(k):
            return mix4[:, k, :, :]          # (128, NT, M)

        def oc(k):
            return out2[:, :, k]             # (128, NT)

        n1b = scal_sb[:, 0:NT]
        n2b = scal_sb[:, NT:2 * NT]
        ucb = scal_sb[:, 2 * NT:3 * NT]
        upb = scal_sb[:, 3 * NT:4 * NT]

        def ht(shape, name):
            return phead.tile(shape, F32, name=name, tag=name)

        # pi softmax (logits tiny -> no max subtraction needed)
        pi_e = ht([128, NT, M], "pie")
        nc.scalar.activation(pi_e[:], mv2(0), AF.Exp)
        se = ht([128, NT], "se")
        nc.vector.reduce_sum(se[:], pi_e[:], axis=mybir.AxisListType.X)
        rse = ht([128, NT], "rse")
        nc.vector.reciprocal(rse[:], se[:])
        for t in range(NT):
            nc.vector.tensor_scalar_mul(out2[:, t, 5:25], pi_e[:, t, :],
                                        rse[:, t:t + 1])

        nc.vector.tensor_copy(out2[:, :, 25:45], mv2(1))        # mu_x
        nc.vector.tensor_copy(out2[:, :, 45:65], mv2(2))        # mu_y
        nc.scalar.activation(out2[:, :, 65:85], mv2(3), AF.Exp)     # std_x
        nc.scalar.activation(out2[:, :, 85:105], mv2(4), AF.Exp)    # std_y
        nc.scalar.activation(out2[:, :, 105:125], mv2(5), AF.Tanh)  # rho

        # q softmax
        qe = ht([128, NT, 3], "qe")
        nc.scalar.activation(qe[:], head2[:, :, 120:123], AF.Exp)
        sq = ht([128, NT], "sq")
        nc.vector.reduce_sum(sq[:], qe[:], axis=mybir.AxisListType.X)
        rsq = ht([128, NT], "rsq")
        nc.vector.reciprocal(rsq[:], sq[:])
        for t in range(NT):
            nc.vector.tensor_scalar_mul(out2[:, t, 125:128], qe[:, t, :],
                                        rsq[:, t:t + 1])

        # component pick: cumsum(e) < u * sum(e)  (same predicate as
        # cumsum(softmax) < u, fewer roundings)
        cA = ht([128, NT, M], "cA")
        cB = ht([128, NT, M], "cB")
        nc.vector.tensor_copy(cA[:], pi_e[:])
        src, dst = cA, cB
        for k in (1, 2, 4, 8, 16):
            nc.vector.tensor_copy(dst[:, :, 0:k], src[:, :, 0:k])
            nc.vector.tensor_add(dst[:, :, k:M], src[:, :, k:M],
                                 src[:, :, 0:M - k])
            src, dst = dst, src
        cum = src
        use_ = ht([128, NT], "use_")
        nc.vector.tensor_mul(use_[:], ucb, se[:])

        Ind = ht([128, NT, M], "Ind")
        for t in range(NT):
            nc.vector.tensor_scalar(Ind[:, t, :], cum[:, t, :],
                                    use_[:, t:t + 1], None, op0=OP.is_lt)
        oh = ht([128, NT, M], "oh")
        nc.vector.tensor_scalar(oh[:, :, 0:1], Ind[:, :, 0:1], -1.0, 1.0,
                                op0=OP.mult, op1=OP.add)
        nc.vector.tensor_sub(oh[:, :, 1:M - 1], Ind[:, :, 0:M - 2],
                             Ind[:, :, 1:M - 1])
        nc.vector.tensor_copy(oh[:, :, M - 1:M], Ind[:, :, M - 2:M - 1])

        # gathers: take(a) = sum_j oh_j * a_j  -> (128, NT)
        takes = {}
        for name, src_ap in (
            ("mx", out2[:, :, 25:45]), ("my", out2[:, :, 45:65]),
            ("sx", out2[:, :, 65:85]), ("sy", out2[:, :, 85:105]),
            ("r", out2[:, :, 105:125]), ("tv", mv2(5)),
        ):
            scr = ht([128, NT, M], f"scr_{name}")
            dst_s = ht([128, NT], f"tk_{name}")
            nc.vector.tensor_mul(scr[:], oh[:], src_ap)
            nc.vector.reduce_sum(dst_s[:], scr[:], axis=mybir.AxisListType.X)
            takes[name] = dst_s

        # x_s = mx + sx * n1
        t1 = ht([128, NT], "t1")
        nc.vector.tensor_mul(t1[:], takes["sx"][:], n1b)
        nc.vector.tensor_add(oc(0), takes["mx"][:], t1[:])

        # y_s = my + sy * (r*n1 + sech(v)*n2), sech(v) = sqrt(1 - tanh(v)^2)
        ev = ht([128, NT], "ev")
        nc.scalar.activation(ev[:], takes["tv"][:], AF.Exp)
        rev = ht([128, NT], "rev")
        nc.vector.reciprocal(rev[:], ev[:])
        den = ht([128, NT], "den")
        nc.vector.tensor_add(den[:], ev[:], rev[:])
        rden = ht([128, NT], "rden")
        nc.vector.reciprocal(rden[:], den[:])
        tt = ht([128, NT], "tt")
        nc.vector.tensor_mul(tt[:], rden[:], n2b)
        tt2 = ht([128, NT], "tt2")
        nc.vector.tensor_scalar_mul(tt2[:], tt[:], 2.0)
        rn1 = ht([128, NT], "rn1")
        nc.vector.tensor_mul(rn1[:], takes["r"][:], n1b)
        inner = ht([128, NT], "inner")
        nc.vector.tensor_add(inner[:], tt2[:], rn1[:])
        t4 = ht([128, NT], "t4")
        nc.vector.tensor_mul(t4[:], takes["sy"][:], inner[:])
        nc.vector.tensor_add(oc(1), takes["my"][:], t4[:])

        # pen one-hot from cumsum(q) < u_pen
        cq = ht([128, NT, 3], "cq")
        nc.vector.tensor_copy(cq[:, :, 0:1], out2[:, :, 125:126])
        nc.vector.tensor_add(cq[:, :, 1:2], out2[:, :, 125:126],
                             out2[:, :, 126:127])
        nc.vector.tensor_add(cq[:, :, 2:3], cq[:, :, 1:2], out2[:, :, 127:128])
        Iq = ht([128, NT, 3], "Iq")
        for t in range(NT):
            nc.vector.tensor_scalar(Iq[:, t, :], cq[:, t, :], upb[:, t:t + 1],
                                    None, op0=OP.is_lt)
        nc.vector.tensor_scalar(out2[:, :, 2:3], Iq[:, :, 0:1], -1.0, 1.0,
                                op0=OP.mult, op1=OP.add)
        nc.vector.tensor_sub(out2[:, :, 3:4], Iq[:, :, 0:1], Iq[:, :, 1:2])
        nc.vector.tensor_copy(out2[:, :, 4:5], Iq[:, :, 1:2])

        nc.sync.dma_start(
            out_d.rearrange("(t p) c -> p t c", p=128), out2[:])

    nc.compile()
    return nc


def _get_nc(general: bool):
    mm = os.environ.get("KDTYPE", "f32r")
    mm_dt = mybir.dt.float32r if mm == "f32r" else mybir.dt.float32
    key = (general, mm)
    if key not in _BUILT:
        _BUILT[key] = _build(general, mm_dt)
    return _BUILT[key]


def _tf32_round(a: np.ndarray) -> np.ndarray:
    """Round fp32 to TF32 (10-bit mantissa) with round-to-nearest-even.

    fp32r on the PE is TF32; pre-rounding on host makes the result
    independent of whether the hardware rounds or truncates on read.
    """
    u = a.view(np.uint32)
    r = (u + 0xFFF + ((u >> 13) & 1)) & np.uint32(0xFFFFE000)
    return r.view(np.float32)


def _host_inputs(general, z, stroke, hidden, cell, W_ih, W_hh, b_ih, b_hh,
                 W_proj, b_proj, n1, n2, u_comp, u_pen):
    """Build the per-core input maps (shared weight arrays, sharded batch)."""
    f = np.float32
    bias = (b_ih + b_hh).astype(f)
    if general:
        W_aug = np.concatenate(
            [W_ih, W_hh, bias[:, None]], axis=1, dtype=f)          # (8192, 2182)
        x_aug = np.concatenate(
            [z, stroke, hidden, np.ones((B, 1), f)], axis=1, dtype=f)
        block_ids = (0, 1, 2, 3)
    else:
        W_aug = np.concatenate([W_ih, bias[:, None]], axis=1, dtype=f)  # (8192, 134)
        x_aug = np.concatenate([z, stroke, np.ones((B, 1), f)], axis=1, dtype=f)
        block_ids = (0, 2, 3)   # i, g, o

    # interleave gate blocks by H tile: m-tile order [i_h, (f_h,) g_h, o_h]
    rows = []
    for h in range(NH):
        for b in block_ids:
            rows.append(np.arange(b * H + h * 128, b * H + (h + 1) * 128))
    rows = np.concatenate(rows)
    wT = np.ascontiguousarray(W_aug[rows].T)                  # (KX, NG*128)
    wpT = np.ascontiguousarray(
        np.concatenate([W_proj, b_proj[:, None]], axis=1, dtype=f).T)  # (2049,123)
    if os.environ.get("KDTYPE", "f32r") == "f32r":
        wT = _tf32_round(wT)
        wpT = _tf32_round(wpT)
        x_aug = _tf32_round(np.ascontiguousarray(x_aug))

    svals = np.stack([n1, n2, u_comp, u_pen], axis=-1).astype(f)  # (B, 4)

    in_maps = []
    for c in range(NCORES):
        sl = slice(c * BL, (c + 1) * BL)
        m = {
            "xT": np.ascontiguousarray(x_aug[sl].T),
            "ones": np.ones((1, BL), f),
            "wT": wT,
            "wprojT": wpT,
            # scal[p, k, t]: k in (n1, n2, u_comp, u_pen), t = batch tile
            "scal": np.ascontiguousarray(
                svals[sl].reshape(BL // 128, 128, 4).transpose(1, 2, 0)
                .reshape(128, 8)),
        }
        if general:
            m["cT"] = np.ascontiguousarray(cell[sl].T.astype(f))
        in_maps.append(m)
    return in_maps


def kernel(z, stroke, hidden, cell, W_ih, W_hh, b_ih, b_hh, W_proj, b_proj,
           n1, n2, u_comp, u_pen):
    global LAST
    args = [np.asarray(a, dtype=np.float32) for a in (
        z, stroke, hidden, cell, W_ih, W_hh, b_ih, b_hh, W_proj, b_proj,
        n1, n2, u_comp, u_pen)]
    (z, stroke, hidden, cell, W_ih, W_hh, b_ih, b_hh, W_proj, b_proj,
     n1, n2, u_comp, u_pen) = args

    general = bool(hidden.any() or cell.any())
    nc = _get_nc(general)
    in_maps = _host_inputs(general, z, stroke, hidden, cell, W_ih, W_hh,
                           b_ih, b_hh, W_proj, b_proj, n1, n2, u_comp, u_pen)

    trace = os.environ.get("KTRACE", "0") == "1"
    LAST = run_bass_kernel_spmd(nc, in_maps, list(range(NCORES)), trace=trace)
    out = np.concatenate([r["out_pack"] for r in LAST.results], axis=0)

    stroke_next = np.ascontiguousarray(out[:, 0:5])
    pi = np.ascontiguousarray(out[:, 5:25])
    mu_x = np.ascontiguousarray(out[:, 25:45])
    mu_y = np.ascontiguousarray(out[:, 45:65])
    std_x = np.ascontiguousarray(out[:, 65:85])
    std_y = np.ascontiguousarray(out[:, 85:105])
    rho = np.ascontiguousarray(out[:, 105:125])
    q = np.ascontiguousarray(out[:, 125:128])
    return (stroke_next, pi, mu_x, mu_y, std_x, std_y, rho, q)
